# revision 31
# baseline (speedup 1.0000x reference)
"""Per-pixel dynamic 5x5 conv (KernelConv2d) + leaky-relu, data-parallel on 8 TRN2 cores.

Sharding: core i <- (n = i//2, h-half = i%2); each core computes out[n, :, h0:h0+128, :].

v3 design (TensorE-accumulate, column-halved):
- DVE computes ONLY the 25 per-tap elementwise products (x-window * kernel
  plane) in x-row partition space; the dy partition-shift and the 25-tap sum
  run on TensorE as shifted-identity matmuls accumulating in fp32 PSUM
  (ldweights skipped when consecutive matmuls share weights). ScalarE scales
  0.2*x out of PSUM, DVE finishes leaky-relu with max(0.2x, x).
- x is loaded ONCE (plus a 1-col-shifted copy for 4B alignment of odd dx);
  no 5x row duplication: ~15.3MB HBM per core vs baseline ~19MB.
- The entire pipeline is split into two column halves (w<128 / w>=128): the
  kernel stream delivers all taps of half 0 first, so half 0 accumulates,
  evicts, and DMAs out while half 1 is still streaming -- the post-stream
  tail is only half 1's last tap + a short evict chain.
- Output rows p with p+dy > 127 (x rows 128..131) are covered by a
  50-partition host-gathered tail product + one scatter matmul per half.

Partition layout: partition q = padded x row h0+q (q=0..127). Product plane for
tap (dy,dx): P[q] = x[q, w+dx] * k[dy,dx][row h0+q-dy] (kernel plane pre-shifted
host-side, zero rows where q<dy). Shift_dy[q, p] = 1 iff p == q-dy routes row q
to output row p and matmul-accumulates over taps.
"""

import os
from contextlib import ExitStack

import numpy as np

import concourse.bass as bass
import concourse.mybir as mybir
from concourse.bass_utils import run_bass_kernel_spmd

N, C, H, W = 4, 8, 256, 256
K = 5
PAD = 2
NCORES = 8
HSH = H // 2            # 128 output rows per core
XW = 264                # stored x row width per alignment copy
HW2 = W // 2            # 128: column half width
CD = mybir.dt.float16
NEG = 0.2
NB = 8                  # DVE product ring buffers
NG = 3                  # GpSimd product ring buffers
NWARM = 6               # PE warm-up dummy matmuls (HAM clock-gate)
NTAP = K * K            # 25
NPAIR = 13              # kernel DMA granularity: 2 taps per DMA per half
COMBOS = [(dy, p) for dy in (1, 2, 3, 4) for p in range(HSH - dy, HSH)]
NTAIL = len(COMBOS) * K  # 50
NSEQ = 2 * NTAP         # 50 half-tap products
# products handled by GpSimd (spread through the stream, none near the end)
GSET = frozenset(s for s in range(2, 47, 4))  # 2,6,...,46 -> 12 products
VSEQ = [s for s in range(NSEQ) if s not in GSET]
GSEQ = [s for s in range(NSEQ) if s in GSET]

_NC_CACHE = {}


def _build_nc():
    nc = bass.Bass("TRN2", target_bir_lowering=False, debug=False,
                   num_devices=NCORES)
    # aux: x windows (2 alignments) + the 6 shift/scatter matrices
    aux_d = nc.dram_tensor("aux", [HSH, 2 * C * XW + 6 * HSH], CD,
                           kind="ExternalInput").ap()
    tl_d = nc.dram_tensor("tl", [NTAIL, 2, C, W], CD, kind="ExternalInput").ap()
    kp_d = nc.dram_tensor("kp", [HSH, 2, NTAP, C, HW2], CD,
                          kind="ExternalInput").ap()
    out_d = nc.dram_tensor("out", [HSH, 2, C, HW2], CD, kind="ExternalOutput").ap()

    with ExitStack() as ctx:
        aux = ctx.enter_context(
            nc.sbuf_tensor("aux_s", [HSH, 2 * C * XW + 6 * HSH], CD))
        tl = ctx.enter_context(nc.sbuf_tensor("tl_s", [NTAIL, 2, C, W], CD))
        kt = ctx.enter_context(nc.sbuf_tensor("kt_s", [HSH, 2, NTAP, C, HW2], CD))
        prod = [ctx.enter_context(nc.sbuf_tensor(f"pr{b}", [HSH, C, HW2], CD))
                for b in range(NB)]
        prodg = [ctx.enter_context(nc.sbuf_tensor(f"pg{b}", [HSH, C, HW2], CD))
                 for b in range(NG)]
        ptail = ctx.enter_context(nc.sbuf_tensor("ptail", [NTAIL, 2, C, HW2], CD))
        tmp = ctx.enter_context(nc.sbuf_tensor("tmp", [HSH, 2, C, HW2], CD))
        ot = ctx.enter_context(nc.sbuf_tensor("ot", [HSH, 2, C, HW2], CD))
        pt = ctx.enter_context(
            nc.psum_tensor("pt", [HSH, 2, C, HW2], mybir.dt.float32))
        scr = ctx.enter_context(
            nc.psum_tensor("scr", [HSH, 512], mybir.dt.float32))

        xs = aux[:, 0:2 * C * XW].rearrange("p (a c x) -> p a c x", a=2, c=C)
        wt = aux[:, 2 * C * XW:].rearrange("p (g m) -> p g m", g=6)

        s_a = ctx.enter_context(nc.semaphore("s_a"))
        s_tl = ctx.enter_context(nc.semaphore("s_tl"))
        s_k = [ctx.enter_context(nc.semaphore(f"s_k{j}"))
               for j in range(2 * NPAIR)]
        s_v = ctx.enter_context(nc.semaphore("s_v"))    # DVE products done
        s_g = ctx.enter_context(nc.semaphore("s_g"))    # GpSimd products done
        s_mm = ctx.enter_context(nc.semaphore("s_mm"))  # PE tap-groups done
        s_c = ctx.enter_context(nc.semaphore("s_c"))    # per-half final MMs
        s_t = ctx.enter_context(nc.semaphore("s_t"))    # 0.2*x chunks done
        s_e = ctx.enter_context(nc.semaphore("s_e"))    # lrelu chunks done
        s_o = ctx.enter_context(nc.semaphore("s_o"))
        block = ctx.enter_context(nc.Block())

        # 50 half-tap products (h0 taps then h1), split DVE/GpSimd by GSET;
        # PE consumes products in pairs (t = seq//2); s_mm hits t+2 when pair t
        # is consumed (pair 24 increments s_c instead, never waited via s_mm).
        def smm_after(j):  # s_mm value guaranteeing half-tap seq j is consumed
            t = j // 2
            assert t < 24
            return t + 2

        def v_count(s):  # DVE-produced seqs <= s
            return sum(1 for x in VSEQ if x <= s)

        def g_count(s):  # GpSimd-produced seqs <= s
            return sum(1 for x in GSEQ if x <= s)

        def tap_geom(seq):
            h, i = divmod(seq, NTAP)
            dy, dx = divmod(i, K)
            a = dx & 1
            off = dx - a + h * HW2
            return h, i, dy, a, off

        @block.sync
        def _(sync):
            sync.dma_start(tl[:], tl_d).then_inc(s_tl, 16)
            sync.dma_start(aux[:], aux_d).then_inc(s_a, 16)
            for h in range(2):
                for j in range(NPAIR):
                    t0, t1 = 2 * j, min(2 * j + 2, NTAP)
                    sync.dma_start(kt[:, h, t0:t1],
                                   kp_d[:, h, t0:t1]).then_inc(
                                       s_k[h * NPAIR + j], 16)
            sync.wait_ge(s_o, 48)

        @block.vector
        def _(vector):
            vector.wait_ge(s_tl, 16)
            for h in range(2):
                vector.tensor_tensor(ptail[0:NTAIL, h],
                                     tl[0:NTAIL, 0, :, h * HW2:(h + 1) * HW2],
                                     tl[0:NTAIL, 1, :, h * HW2:(h + 1) * HW2],
                                     op=mybir.AluOpType.mult).then_inc(s_v, 1)
            vector.wait_ge(s_a, 16)

            def lrelu_max(vector, h, q):
                vector.wait_ge(s_t, 2 * h + q + 1)
                vector.tensor_tensor(ot[:, h, 4 * q:4 * q + 4],
                                     tmp[:, h, 4 * q:4 * q + 4],
                                     pt[:, h, 4 * q:4 * q + 4],
                                     op=mybir.AluOpType.max).then_inc(s_e, 1)

            last_k = [-1]
            for vidx, seq in enumerate(VSEQ):
                h, i, dy, a, off = tap_geom(seq)
                pair_dma = h * NPAIR + i // 2
                if pair_dma != last_k[0]:
                    vector.wait_ge(s_k[pair_dma], 16)
                    last_k[0] = pair_dma
                if vidx >= NB and vidx % 4 == 0:
                    # batched ring-reuse wait covering vidx..vidx+3
                    vector.wait_ge(s_mm, smm_after(VSEQ[vidx - 5]))
                vector.tensor_tensor(prod[vidx % NB][:],
                                     xs[:, a, :, off:off + HW2],
                                     kt[:, h, i],
                                     op=mybir.AluOpType.mult).then_inc(s_v, 1)
                # half-0 lrelu slots mid-way through half-1's stream
                # (seqs chosen to be VSEQ members: not ==2 mod 4)
                if seq == NTAP + 3:
                    lrelu_max(vector, 0, 0)
                if seq == NTAP + 6:
                    lrelu_max(vector, 0, 1)
            lrelu_max(vector, 1, 0)
            lrelu_max(vector, 1, 1)

        @block.gpsimd
        def _(gpsimd):
            gpsimd.wait_ge(s_a, 16)
            last_k = [-1]
            for gidx, seq in enumerate(GSEQ):
                h, i, dy, a, off = tap_geom(seq)
                pair_dma = h * NPAIR + i // 2
                if pair_dma != last_k[0]:
                    gpsimd.wait_ge(s_k[pair_dma], 16)
                    last_k[0] = pair_dma
                if gidx >= NG:
                    gpsimd.wait_ge(s_mm, smm_after(GSEQ[gidx - NG]))
                gpsimd.tensor_tensor(prodg[gidx % NG][:],
                                     xs[:, a, :, off:off + HW2],
                                     kt[:, h, i],
                                     op=mybir.AluOpType.mult).then_inc(s_g, 1)

        @block.tensor
        def _(tensor):
            tensor.wait_ge(s_a, 16)
            # dummy matmuls: engage the PE HAM clock-gate (~3.4us of activity
            # flips the PE from 1.2 to 2.4 GHz) before the real stream arrives
            for r in range(NWARM):
                mm = tensor.matmul(scr[:], lhsT=wt[:, 0],
                                   rhs=xs[:, 0, 0:2, 0:W],
                                   start=True, stop=True)
                if r > 0:
                    mm.ins.ldweights = False
            prev_w = [0]  # warmup loaded wt[:, 0]

            def mmul(rhs_ap, h, q, wid, start, stop):
                mm = tensor.matmul(pt[:, h, 4 * q:4 * q + 4],
                                   lhsT=(wt[0:NTAIL, 5] if wid == 5
                                         else wt[:, wid]),
                                   rhs=rhs_ap,
                                   start=start, stop=stop)
                if wid == prev_w[0]:
                    mm.ins.ldweights = False
                prev_w[0] = wid
                return mm

            tensor.wait_ge(s_v, 2)
            for h in range(2):
                for q in range(2):
                    mm = mmul(ptail[0:NTAIL, h, 4 * q:4 * q + 4], h, q, 5,
                              True, False)
            mm.then_inc(s_mm, 1)
            lastw = [0, 0]  # last waited (s_v, s_g) values
            for t in range(NTAP):  # pair t covers seqs 2t, 2t+1
                b = 2 * t + 1
                nv, ng = 2 + v_count(b), g_count(b)
                if nv > lastw[0]:
                    tensor.wait_ge(s_v, nv)
                    lastw[0] = nv
                if ng > lastw[1]:
                    tensor.wait_ge(s_g, ng)
                    lastw[1] = ng
                pair_last = None
                for seq in (2 * t, 2 * t + 1):
                    h, i = divmod(seq, NTAP)
                    dy = i // K
                    last = i == NTAP - 1
                    if seq in GSET:
                        rbuf = prodg[GSEQ.index(seq) % NG]
                    else:
                        rbuf = prod[VSEQ.index(seq) % NB]
                    for q in range(2):
                        mm = mmul(rbuf[:, 4 * q:4 * q + 4],
                                  h, q, dy, False, last)
                        if last and q == 1:
                            mm.then_inc(s_c, 1)
                        elif not last:
                            pair_last = mm
                if pair_last is not None and t < NTAP - 1:
                    pair_last.then_inc(s_mm, 1)

        @block.scalar
        def _(scalar):
            # acts + output DMAs ride the ACT HWDGE ring, which is empty --
            # issuing outputs from sync would FIFO behind the whole input stream
            scalar.wait_ge(s_a, 16)
            # preload the activation table outside the critical path
            scalar.activation(tmp[:, 0, 0], xs[:, 0, 0, 0:HW2],
                              mybir.ActivationFunctionType.Copy,
                              bias=0.0, scale=1.0)
            for h in range(2):
                for q in range(2):
                    scalar.wait_ge(s_c, h + 1)
                    scalar.activation(tmp[:, h, 4 * q:4 * q + 4],
                                      pt[:, h, 4 * q:4 * q + 4],
                                      mybir.ActivationFunctionType.Copy,
                                      bias=0.0, scale=NEG).then_inc(s_t, 1)
            scalar.wait_ge(s_e, 2)
            scalar.dma_start(out_d[:, 0], ot[:, 0]).then_inc(s_o, 16)
            # half 1 is the critical tail: ship each lrelu chunk as it lands
            scalar.wait_ge(s_e, 3)
            scalar.dma_start(out_d[:, 1, 0:4], ot[:, 1, 0:4]).then_inc(s_o, 16)
            scalar.wait_ge(s_e, 4)
            scalar.dma_start(out_d[:, 1, 4:8], ot[:, 1, 4:8]).then_inc(s_o, 16)
    return nc


def get_nc():
    if "nc" not in _NC_CACHE:
        _NC_CACHE["nc"] = _build_nc()
    return _NC_CACHE["nc"]


def _prep_shards(x: np.ndarray, kernel: np.ndarray):
    """Host-side: pad, cast to fp16, build per-core DMA layouts."""
    f16 = np.float16
    xp = np.pad(x, ((0, 0), (0, 0), (PAD, PAD), (PAD, XW + 1 - W - PAD)),
                mode='edge').astype(f16)  # (N, C, 260, 265)
    kr = kernel.reshape(N, C, NTAP, H, W)

    in_maps = []
    for core in range(NCORES):
        n, hb = divmod(core, 2)
        h0 = hb * HSH
        blk = xp[n, :, h0:h0 + HSH + 4, :]          # (C, 132, 265)
        aux = np.zeros((HSH, 2 * C * XW + 6 * HSH), f16)
        xsb = aux[:, :2 * C * XW].reshape(HSH, 2, C, XW)
        xsb[:, 0] = blk[:, :HSH, 0:XW].transpose(1, 0, 2)
        xsb[:, 1] = blk[:, :HSH, 1:XW + 1].transpose(1, 0, 2)
        wtb = aux[:, 2 * C * XW:].reshape(HSH, 6, HSH)
        for dy in range(K):
            q = np.arange(dy, HSH)
            wtb[q, dy, q - dy] = 1.0
        for j, (dy, p) in enumerate(COMBOS):
            for dx in range(K):
                wtb[j * K + dx, 5, p] = 1.0

        kb = kr[n, :, :, h0:h0 + HSH, :].astype(f16)  # (C, 25, 128, W)
        kp = np.zeros((HSH, NTAP, C, W), f16)
        for t in range(NTAP):
            dy = t // K
            kp[dy:, t] = kb[:, t, :HSH - dy].transpose(1, 0, 2)
        # column-half-major: (q, half, tap, c, 128)
        kph = np.ascontiguousarray(
            kp.reshape(HSH, NTAP, C, 2, HW2).transpose(0, 3, 1, 2, 4))

        tlb = np.zeros((NTAIL, 2, C, W), f16)
        for j, (dy, p) in enumerate(COMBOS):
            for dx in range(K):
                tlb[j * K + dx, 0] = blk[:, p + dy, dx:dx + W]
                tlb[j * K + dx, 1] = kb[:, dy * K + dx, p]

        in_maps.append({"aux": aux, "tl": tlb, "kp": kph})
    return in_maps


def kernel(x: np.ndarray, kernel: np.ndarray) -> np.ndarray:
    nc = get_nc()
    in_maps = _prep_shards(np.asarray(x), np.asarray(kernel))
    trace = bool(int(os.environ.get("KC_TRACE", "0")))
    res = run_bass_kernel_spmd(nc, in_maps, core_ids=list(range(NCORES)),
                               trace=trace)
    _NC_CACHE["last_results"] = res
    out = np.empty((N, C, H, W), np.float32)
    for core in range(NCORES):
        n, hb = divmod(core, 2)
        h0 = hb * HSH
        o = res.results[core]["out"]  # (128, 2, C, 128) fp16
        o = o.transpose(2, 0, 1, 3).reshape(C, HSH, W)
        out[n, :, h0:h0 + HSH, :] = o.astype(np.float32)
    return out


# revision 32
# speedup vs baseline: 1.1672x; 1.1672x over previous
"""Per-pixel dynamic 5x5 conv (KernelConv2d) + leaky-relu, data-parallel on 8 TRN2 cores.

Sharding: core i <- (n = i//2, h-half = i%2); each core computes out[n, :, h0:h0+128, :].

v7 design (TensorE-accumulate, full-width products):
- DVE computes ONLY the 25 per-tap elementwise products (x-window * kernel
  plane) in x-row partition space; the dy partition-shift and the 25-tap sum
  run on the otherwise-idle TensorE as shifted-identity matmuls accumulating
  in fp32 PSUM (ldweights skipped when consecutive matmuls share weights).
- x loaded ONCE, even-alignment only; the 1-col-shifted odd-alignment copy
  (for 4B-aligned odd-dx reads at DVE 2x mode) is built on-chip by ScalarE.
  HBM traffic ~14.3MB/core vs baseline ~19MB.
- PE warm-up: dummy matmuls on garbage SBUF right after the preamble flip the
  HAM clock-gate to 2.4GHz before real work arrives.
- Eviction: per 512-col PSUM chunk, stop-matmul -> ScalarE 0.2*x -> DVE
  max(0.2x, x) -> chunked output DMA on the ACT HWDGE ring (the sync ring
  would FIFO the output behind the whole input stream).
- Output rows p with p+dy > 127 (x rows 128..131) are covered by a
  50-partition host-gathered tail product + one scatter matmul, mid-queue.

Partition layout: partition q = padded x row h0+q (q=0..127). Product plane for
tap (dy,dx): P[q] = x[q, w+dx] * k[dy,dx][row h0+q-dy] (kernel plane pre-shifted
host-side, zero rows where q<dy). Shift_dy[q, p] = 1 iff p == q-dy routes row q
to output row p and matmul-accumulates over taps.
"""

import os
from contextlib import ExitStack

import numpy as np

import concourse.bass as bass
import concourse.mybir as mybir
from concourse.bass_utils import run_bass_kernel_spmd

N, C, H, W = 4, 8, 256, 256
K = 5
PAD = 2
NCORES = 8
HSH = H // 2            # 128 output rows per core
XW = 264                # stored x row width per alignment copy
CD = mybir.dt.float16
NEG = 0.2
NB = 8                  # product ring buffers
NWARM = 12              # PE warm-up dummy matmuls (HAM clock-gate)
NTAP = K * K            # 25
COMBOS = [(dy, p) for dy in (1, 2, 3, 4) for p in range(HSH - dy, HSH)]
NTAIL = len(COMBOS) * K  # 50
TAILPOS = 13            # queue position of the tail product (after product 12)

_NC_CACHE = {}


def _qpos(p):  # queue position of product p (tail occupies TAILPOS)
    return p if p < TAILPOS else p + 1


def _build_nc():
    nc = bass.Bass("TRN2", target_bir_lowering=False, debug=False,
                   num_devices=NCORES)
    xe_d = nc.dram_tensor("xe", [HSH, C, XW], CD, kind="ExternalInput").ap()
    wt_d = nc.dram_tensor("wt", [HSH, 6, HSH], CD, kind="ExternalInput").ap()
    tl_d = nc.dram_tensor("tl", [NTAIL, 2, C, W], CD, kind="ExternalInput").ap()
    kp_d = nc.dram_tensor("kp", [HSH, NTAP, C, W], CD, kind="ExternalInput").ap()
    out_d = nc.dram_tensor("out", [HSH, C, W], CD, kind="ExternalOutput").ap()

    with ExitStack() as ctx:
        xe = ctx.enter_context(nc.sbuf_tensor("xe_s", [HSH, C, XW], CD))
        xo = ctx.enter_context(nc.sbuf_tensor("xo_s", [HSH, C, XW], CD))
        wt = ctx.enter_context(nc.sbuf_tensor("wt_s", [HSH, 6, HSH], CD))
        tl = ctx.enter_context(nc.sbuf_tensor("tl_s", [NTAIL, 2, C, W], CD))
        kt = ctx.enter_context(nc.sbuf_tensor("kt_s", [HSH, NTAP, C, W], CD))
        prod = [ctx.enter_context(nc.sbuf_tensor(f"pr{b}", [HSH, C, W], CD))
                for b in range(NB)]
        ptail = ctx.enter_context(nc.sbuf_tensor("ptail", [NTAIL, C, W], CD))
        tmp = ctx.enter_context(nc.sbuf_tensor("tmp", [HSH, C, W], CD))
        ot = ctx.enter_context(nc.sbuf_tensor("ot", [HSH, C, W], CD))
        pt = ctx.enter_context(nc.psum_tensor("pt", [HSH, C, W], mybir.dt.float32))
        scr = ctx.enter_context(nc.psum_tensor("scr", [HSH, 512], mybir.dt.float32))

        s_xe = ctx.enter_context(nc.semaphore("s_xe"))
        s_w = ctx.enter_context(nc.semaphore("s_w"))
        s_tl = ctx.enter_context(nc.semaphore("s_tl"))
        s_k = [ctx.enter_context(nc.semaphore(f"s_k{j}")) for j in range(13)]
        s_x2 = ctx.enter_context(nc.semaphore("s_x2"))  # odd copy done
        s_v = ctx.enter_context(nc.semaphore("s_v"))    # queue items produced
        s_mm = ctx.enter_context(nc.semaphore("s_mm"))  # queue items consumed
        s_c = ctx.enter_context(nc.semaphore("s_c"))    # per-chunk stop MMs
        s_t = ctx.enter_context(nc.semaphore("s_t"))    # 0.2*x chunks done
        s_e = ctx.enter_context(nc.semaphore("s_e"))    # lrelu chunks done
        s_o = ctx.enter_context(nc.semaphore("s_o"))
        block = ctx.enter_context(nc.Block())

        # kernel-plane DMAs: j=0 -> tap 0 alone (short critical head), then
        # pairs (2j-1, 2j). Product tap t needs DMA (t+1)//2.
        def kdma(j):
            return (0, 1) if j == 0 else (2 * j - 1, 2 * j + 1)

        @block.sync
        def _(sync):
            sync.dma_start(xe[:], xe_d).then_inc(s_xe, 16)
            sync.dma_start(wt[:], wt_d).then_inc(s_w, 16)
            for j in range(13):
                t0, t1 = kdma(j)
                sync.dma_start(kt[:, t0:t1], kp_d[:, t0:t1]).then_inc(s_k[j], 16)
                if j == 6:
                    sync.dma_start(tl[:], tl_d).then_inc(s_tl, 16)
            sync.wait_ge(s_o, 64)

        @block.vector
        def _(vector):
            vector.wait_ge(s_xe, 16)
            for p in range(NTAP):
                dy, dx = divmod(p, K)
                a = dx & 1
                xsrc = xo if a else xe
                off = dx - a
                if p == 1:
                    vector.wait_ge(s_x2, 1)
                j = (p + 1) // 2
                if p == 0 or p % 2 == 1:
                    vector.wait_ge(s_k[j], 16)
                if p >= NB and p % 4 == 0:
                    # batched ring-reuse wait covering products p..p+3
                    vector.wait_ge(s_mm, _qpos(p - 5) + 1)
                vector.tensor_tensor(prod[p % NB][:],
                                     xsrc[:, :, off:off + W],
                                     kt[:, p],
                                     op=mybir.AluOpType.mult).then_inc(s_v, 1)
                if p == TAILPOS - 1:  # tail product right after product 12
                    vector.wait_ge(s_tl, 16)
                    vector.tensor_tensor(ptail[0:NTAIL],
                                         tl[0:NTAIL, 0],
                                         tl[0:NTAIL, 1],
                                         op=mybir.AluOpType.mult).then_inc(s_v, 1)
            for q in range(4):
                vector.wait_ge(s_t, q + 1)
                vector.tensor_tensor(ot[:, 2 * q:2 * q + 2],
                                     tmp[:, 2 * q:2 * q + 2],
                                     pt[:, 2 * q:2 * q + 2],
                                     op=mybir.AluOpType.max).then_inc(s_e, 1)

        @block.tensor
        def _(tensor):
            # warm-up on garbage SBUF (ot is only written much later, and that
            # write is sem-ordered after these reads) -- no DMA dependency, so
            # the PE is busy right out of the preamble and HAM unthrottles
            for r in range(NWARM):
                mm = tensor.matmul(scr[:], lhsT=ot[:, 0, 0:HSH],
                                   rhs=ot[:, 0:2, :], start=True, stop=True)
                if r > 0:
                    mm.ins.ldweights = False
            tensor.wait_ge(s_w, 16)
            prev_w = [-1]

            def mmul(rhs_ap, q, wid, start, stop):
                mm = tensor.matmul(pt[:, 2 * q:2 * q + 2],
                                   lhsT=(wt[0:NTAIL, 5] if wid == 5
                                         else wt[:, wid]),
                                   rhs=rhs_ap,
                                   start=start, stop=stop)
                if wid == prev_w[0]:
                    mm.ins.ldweights = False
                prev_w[0] = wid
                return mm

            for n in range(NTAP + 1):  # queue: products + tail at TAILPOS
                tensor.wait_ge(s_v, n + 1)
                if n == TAILPOS:
                    for q in range(4):
                        mm = mmul(ptail[0:NTAIL, 2 * q:2 * q + 2], q, 5,
                                  False, False)
                    mm.then_inc(s_mm, 1)
                    continue
                p = n if n < TAILPOS else n - 1
                dy = p // K
                first, last = p == 0, p == NTAP - 1
                for q in range(4):
                    mm = mmul(prod[p % NB][:, 2 * q:2 * q + 2], q, dy,
                              first, last)
                    if last:
                        mm.then_inc(s_c, 1)
                if not last:
                    mm.then_inc(s_mm, 1)

        @block.scalar
        def _(scalar):
            # odd-alignment x copy: xo[i] = xe[i+1] (cuts 0.54MB off the
            # DMA stream; also preloads the ACT table)
            scalar.wait_ge(s_xe, 16)
            scalar.activation(xo[:, :, 0:XW - 4], xe[:, :, 1:XW - 3],
                              mybir.ActivationFunctionType.Copy,
                              bias=0.0, scale=1.0).then_inc(s_x2, 1)
            # evict: 0.2*x prep, then chunked output DMAs on the ACT ring
            # (sync's ring would FIFO these behind the whole input stream)
            for q in range(4):
                scalar.wait_ge(s_c, q + 1)
                scalar.activation(tmp[:, 2 * q:2 * q + 2],
                                  pt[:, 2 * q:2 * q + 2],
                                  mybir.ActivationFunctionType.Copy,
                                  bias=0.0, scale=NEG).then_inc(s_t, 1)
                if q >= 2:
                    scalar.wait_ge(s_e, q - 1)
                    scalar.dma_start(out_d[:, 2 * (q - 2):2 * (q - 2) + 2],
                                     ot[:, 2 * (q - 2):2 * (q - 2) + 2]
                                     ).then_inc(s_o, 16)
            for q in range(2, 4):
                scalar.wait_ge(s_e, q + 1)
                scalar.dma_start(out_d[:, 2 * q:2 * q + 2],
                                 ot[:, 2 * q:2 * q + 2]).then_inc(s_o, 16)
    return nc


def get_nc():
    if "nc" not in _NC_CACHE:
        _NC_CACHE["nc"] = _build_nc()
    return _NC_CACHE["nc"]


def _prep_shards(x: np.ndarray, kernel: np.ndarray):
    """Host-side: pad, cast to fp16, build per-core DMA layouts."""
    f16 = np.float16
    xp = np.pad(x, ((0, 0), (0, 0), (PAD, PAD), (PAD, XW + 1 - W - PAD)),
                mode='edge').astype(f16)  # (N, C, 260, 265)
    kr = kernel.reshape(N, C, NTAP, H, W)

    in_maps = []
    for core in range(NCORES):
        n, hb = divmod(core, 2)
        h0 = hb * HSH
        blk = xp[n, :, h0:h0 + HSH + 4, :]          # (C, 132, 265)
        xeb = np.ascontiguousarray(blk[:, :HSH, 0:XW].transpose(1, 0, 2))

        wtb = np.zeros((HSH, 6, HSH), f16)
        for dy in range(K):
            q = np.arange(dy, HSH)
            wtb[q, dy, q - dy] = 1.0
        for j, (dy, p) in enumerate(COMBOS):
            for dx in range(K):
                wtb[j * K + dx, 5, p] = 1.0

        kb = kr[n, :, :, h0:h0 + HSH, :].astype(f16)  # (C, 25, 128, W)
        kp = np.zeros((HSH, NTAP, C, W), f16)
        for t in range(NTAP):
            dy = t // K
            kp[dy:, t] = kb[:, t, :HSH - dy].transpose(1, 0, 2)

        tlb = np.zeros((NTAIL, 2, C, W), f16)
        for j, (dy, p) in enumerate(COMBOS):
            for dx in range(K):
                tlb[j * K + dx, 0] = blk[:, p + dy, dx:dx + W]
                tlb[j * K + dx, 1] = kb[:, dy * K + dx, p]

        in_maps.append({"xe": xeb, "wt": wtb, "tl": tlb, "kp": kp})
    return in_maps


def kernel(x: np.ndarray, kernel: np.ndarray) -> np.ndarray:
    nc = get_nc()
    in_maps = _prep_shards(np.asarray(x), np.asarray(kernel))
    trace = bool(int(os.environ.get("KC_TRACE", "0")))
    res = run_bass_kernel_spmd(nc, in_maps, core_ids=list(range(NCORES)),
                               trace=trace)
    _NC_CACHE["last_results"] = res
    out = np.empty((N, C, H, W), np.float32)
    for core in range(NCORES):
        n, hb = divmod(core, 2)
        h0 = hb * HSH
        o = res.results[core]["out"]  # (128, C, W) fp16
        out[n, :, h0:h0 + HSH, :] = o.transpose(1, 0, 2).astype(np.float32)
    return out


# revision 35
# speedup vs baseline: 1.2585x; 1.0782x over previous
"""Per-pixel dynamic 5x5 conv (KernelConv2d) + leaky-relu, data-parallel on 8 TRN2 cores.

Sharding: core i <- (n = i//2, h-half = i%2); each core computes out[n, :, h0:h0+128, :].

v8 design (TensorE-accumulate, full-width products, HBM-roofline stream):
- DVE computes ONLY the 25 per-tap elementwise products (x-window * kernel
  plane) in x-row partition space; the dy partition-shift and the 25-tap sum
  run on the otherwise-idle TensorE as shifted-identity matmuls accumulating
  in fp32 PSUM (ldweights skipped when consecutive matmuls share weights).
- The input stream is at the device HBM roofline (8 cores x ~0.35 B/ns), so
  bytes are trimmed everywhere: x loaded once (even alignment; the odd
  1-col-shifted copy for 4B-aligned odd-dx DVE reads is built on-chip by
  ScalarE), the 5 shift matrices are built on-chip by DVE (memset +
  affine_select) during the DMA head, and the tail scatter matrix rides the
  tl DMA. ~13.5MB HBM per core vs baseline ~19MB.
- PE warm-up: dummy matmuls on garbage SBUF right after the preamble flip the
  HAM clock-gate to 2.4GHz before real work arrives.
- Tail: the last two kernel planes arrive as single-tap DMAs (tap 23's
  product overlaps tap 24's DMA); per 512-col PSUM chunk, stop-matmul ->
  ScalarE 0.2*x -> DVE max(0.2x, x) -> chunked output DMA on the ACT HWDGE
  ring (the sync ring would FIFO the output behind the whole input stream).
- Output rows p with p+dy > 127 (x rows 128..131) are covered by a
  50-partition host-gathered tail product + one scatter matmul, mid-queue.

Partition layout: partition q = padded x row h0+q (q=0..127). Product plane for
tap (dy,dx): P[q] = x[q, w+dx] * k[dy,dx][row h0+q-dy] (kernel plane pre-shifted
host-side, zero rows where q<dy). Shift_dy[q, p] = 1 iff p == q-dy routes row q
to output row p and matmul-accumulates over taps.
"""

import os
from contextlib import ExitStack

import numpy as np

import concourse.bass as bass
import concourse.mybir as mybir
from concourse.bass_utils import run_bass_kernel_spmd

N, C, H, W = 4, 8, 256, 256
K = 5
PAD = 2
NCORES = 8
HSH = H // 2            # 128 output rows per core
XW = 264                # stored x row width per alignment copy
CD = mybir.dt.float16
NEG = 0.2
NB = 8                  # product ring buffers
NWARM = 12              # PE warm-up dummy matmuls (HAM clock-gate)
NTAP = K * K            # 25
COMBOS = [(dy, p) for dy in (1, 2, 3, 4) for p in range(HSH - dy, HSH)]
NTAIL = len(COMBOS) * K  # 50
TAILPOS = 13            # queue position of the tail product (after product 12)
CW = C * W               # 2048
NKD = 14                 # kernel-plane DMAs: [0,1],(1,3),..,(21,23),[23],[24]

_NC_CACHE = {}


def _qpos(p):  # queue position of product p (tail occupies TAILPOS)
    return p if p < TAILPOS else p + 1


def _kdma(j):  # taps [t0, t1) carried by kernel DMA j
    if j == 0:
        return 0, 1
    if j <= 11:
        return 2 * j - 1, 2 * j + 1
    return j + 11, j + 12  # j=12 -> tap 23, j=13 -> tap 24


def _kdma_of(p):  # kernel DMA index carrying tap p
    if p == 0:
        return 0
    if p <= 22:
        return (p + 1) // 2
    return p - 11


def _build_nc():
    nc = bass.Bass("TRN2", target_bir_lowering=False, debug=False,
                   num_devices=NCORES)
    xe_d = nc.dram_tensor("xe", [HSH, C, XW], CD, kind="ExternalInput").ap()
    tl_d = nc.dram_tensor("tl", [NTAIL, 2 * CW + HSH], CD,
                          kind="ExternalInput").ap()
    kp_d = nc.dram_tensor("kp", [HSH, NTAP, C, W], CD, kind="ExternalInput").ap()
    out_d = nc.dram_tensor("out", [HSH, C, W], CD, kind="ExternalOutput").ap()

    with ExitStack() as ctx:
        xe = ctx.enter_context(nc.sbuf_tensor("xe_s", [HSH, C, XW], CD))
        xo = ctx.enter_context(nc.sbuf_tensor("xo_s", [HSH, C, XW], CD))
        wt = ctx.enter_context(nc.sbuf_tensor("wt_s", [HSH, K, HSH], CD))
        tl = ctx.enter_context(nc.sbuf_tensor("tl_s", [NTAIL, 2 * CW + HSH], CD))
        kt = ctx.enter_context(nc.sbuf_tensor("kt_s", [HSH, NTAP, C, W], CD))
        prod = [ctx.enter_context(nc.sbuf_tensor(f"pr{b}", [HSH, C, W], CD))
                for b in range(NB)]
        ptail = ctx.enter_context(nc.sbuf_tensor("ptail", [NTAIL, C, W], CD))
        tmp = ctx.enter_context(nc.sbuf_tensor("tmp", [HSH, C, W], CD))
        ot = ctx.enter_context(nc.sbuf_tensor("ot", [HSH, C, W], CD))
        pt = ctx.enter_context(nc.psum_tensor("pt", [HSH, C, W], mybir.dt.float32))
        scr = ctx.enter_context(nc.psum_tensor("scr", [HSH, 512], mybir.dt.float32))

        xt = tl[:, 0:CW].rearrange("p (c w) -> p c w", c=C)
        ktl = tl[:, CW:2 * CW].rearrange("p (c w) -> p c w", c=C)
        wtl = tl[:, 2 * CW:]                      # [50, 128] scatter matrix

        s_xe = ctx.enter_context(nc.semaphore("s_xe"))
        s_tl = ctx.enter_context(nc.semaphore("s_tl"))
        s_k = [ctx.enter_context(nc.semaphore(f"s_k{j}")) for j in range(NKD)]
        s_w2 = ctx.enter_context(nc.semaphore("s_w2"))  # shift matrices built
        s_x2 = ctx.enter_context(nc.semaphore("s_x2"))  # odd copy done
        s_v = ctx.enter_context(nc.semaphore("s_v"))    # queue items produced
        s_mm = ctx.enter_context(nc.semaphore("s_mm"))  # queue items consumed
        s_c = ctx.enter_context(nc.semaphore("s_c"))    # per-chunk stop MMs
        s_t = ctx.enter_context(nc.semaphore("s_t"))    # 0.2*x chunks done
        s_e = ctx.enter_context(nc.semaphore("s_e"))    # lrelu chunks done
        s_o = ctx.enter_context(nc.semaphore("s_o"))
        block = ctx.enter_context(nc.Block())

        @block.sync
        def _(sync):
            sync.dma_start(xe[:], xe_d).then_inc(s_xe, 16)
            for j in range(NKD):
                t0, t1 = _kdma(j)
                sync.dma_start(kt[:, t0:t1], kp_d[:, t0:t1]).then_inc(s_k[j], 16)
                if j == 6:
                    sync.dma_start(tl[:], tl_d).then_inc(s_tl, 16)
            sync.wait_ge(s_o, 64)

        @block.gpsimd
        def _(gpsimd):
            # build the 5 shift matrices during the DMA head: wt[q, dy, p] =
            # 1 iff p == q - dy  (iota = dy + p - q, select where == 0)
            gpsimd.memset(wt[:], 1.0)
            for dy in range(K):
                sel = gpsimd.affine_select(wt[:, dy], wt[:, dy],
                                           pattern=[[1, HSH]], base=dy,
                                           channel_multiplier=-1,
                                           compare_op=mybir.AluOpType.is_equal,
                                           fill=0.0)
            sel.then_inc(s_w2, 1)

        @block.vector
        def _(vector):
            vector.wait_ge(s_xe, 16)
            for p in range(NTAP):
                dy, dx = divmod(p, K)
                a = dx & 1
                xsrc = xo if a else xe
                off = dx - a
                if p == 1:
                    vector.wait_ge(s_x2, 1)
                if p == 0 or (p % 2 == 1 and p <= 21) or p >= 23:
                    vector.wait_ge(s_k[_kdma_of(p)], 16)
                if p >= NB and p % 4 == 0:
                    # batched ring-reuse wait covering products p..p+3
                    vector.wait_ge(s_mm, _qpos(p - 5) + 1)
                vector.tensor_tensor(prod[p % NB][:],
                                     xsrc[:, :, off:off + W],
                                     kt[:, p],
                                     op=mybir.AluOpType.mult).then_inc(s_v, 1)
                if p == TAILPOS - 1:  # tail product right after product 12
                    vector.wait_ge(s_tl, 16)
                    vector.tensor_tensor(ptail[0:NTAIL], xt[0:NTAIL],
                                         ktl[0:NTAIL],
                                         op=mybir.AluOpType.mult).then_inc(s_v, 1)
            for q in range(4):
                vector.wait_ge(s_t, q + 1)
                vector.tensor_tensor(ot[:, 2 * q:2 * q + 2],
                                     tmp[:, 2 * q:2 * q + 2],
                                     pt[:, 2 * q:2 * q + 2],
                                     op=mybir.AluOpType.max).then_inc(s_e, 1)

        @block.tensor
        def _(tensor):
            # warm-up on garbage SBUF (ot is only written much later, and that
            # write is sem-ordered after these reads) -- no DMA dependency, so
            # the PE is busy right out of the preamble and HAM unthrottles
            for r in range(NWARM):
                mm = tensor.matmul(scr[:], lhsT=ot[:, 0, 0:HSH],
                                   rhs=ot[:, 0:2, :], start=True, stop=True)
                if r > 0:
                    mm.ins.ldweights = False
            tensor.wait_ge(s_w2, 1)
            prev_w = [-1]

            def mmul(rhs_ap, q, wid, start, stop):
                mm = tensor.matmul(pt[:, 2 * q:2 * q + 2],
                                   lhsT=(wtl[0:NTAIL] if wid == 5
                                         else wt[:, wid]),
                                   rhs=rhs_ap,
                                   start=start, stop=stop)
                if wid == prev_w[0]:
                    mm.ins.ldweights = False
                prev_w[0] = wid
                return mm

            for n in range(NTAP + 1):  # queue: products + tail at TAILPOS
                tensor.wait_ge(s_v, n + 1)
                if n == TAILPOS:
                    for q in range(4):
                        mm = mmul(ptail[0:NTAIL, 2 * q:2 * q + 2], q, 5,
                                  False, False)
                    mm.then_inc(s_mm, 1)
                    continue
                p = n if n < TAILPOS else n - 1
                dy = p // K
                first, last = p == 0, p == NTAP - 1
                for q in range(4):
                    mm = mmul(prod[p % NB][:, 2 * q:2 * q + 2], q, dy,
                              first, last)
                    if last:
                        mm.then_inc(s_c, 1)
                if not last:
                    mm.then_inc(s_mm, 1)

        @block.scalar
        def _(scalar):
            # odd-alignment x copy: xo[i] = xe[i+1] (cuts 0.54MB off the
            # DMA stream; also preloads the ACT table)
            scalar.wait_ge(s_xe, 16)
            scalar.activation(xo[:, :, 0:XW - 4], xe[:, :, 1:XW - 3],
                              mybir.ActivationFunctionType.Copy,
                              bias=0.0, scale=1.0).then_inc(s_x2, 1)
            # evict: 0.2*x prep per chunk, then chunked output DMAs on the
            # ACT HWDGE ring (sync's would FIFO behind the whole input stream)
            for q in range(4):
                scalar.wait_ge(s_c, q + 1)
                scalar.activation(tmp[:, 2 * q:2 * q + 2],
                                  pt[:, 2 * q:2 * q + 2],
                                  mybir.ActivationFunctionType.Copy,
                                  bias=0.0, scale=NEG).then_inc(s_t, 1)
            for q in range(4):
                scalar.wait_ge(s_e, q + 1)
                scalar.dma_start(out_d[:, 2 * q:2 * q + 2],
                                 ot[:, 2 * q:2 * q + 2]).then_inc(s_o, 16)
    return nc


def get_nc():
    if "nc" not in _NC_CACHE:
        _NC_CACHE["nc"] = _build_nc()
    return _NC_CACHE["nc"]


def _prep_shards(x: np.ndarray, kernel: np.ndarray):
    """Host-side: pad, cast to fp16, build per-core DMA layouts."""
    f16 = np.float16
    xp = np.pad(x, ((0, 0), (0, 0), (PAD, PAD), (PAD, XW + 1 - W - PAD)),
                mode='edge').astype(f16)  # (N, C, 260, 265)
    kr = kernel.reshape(N, C, NTAP, H, W)

    in_maps = []
    for core in range(NCORES):
        n, hb = divmod(core, 2)
        h0 = hb * HSH
        blk = xp[n, :, h0:h0 + HSH + 4, :]          # (C, 132, 265)
        xeb = np.ascontiguousarray(blk[:, :HSH, 0:XW].transpose(1, 0, 2))

        kb = kr[n, :, :, h0:h0 + HSH, :].astype(f16)  # (C, 25, 128, W)
        kp = np.zeros((HSH, NTAP, C, W), f16)
        for t in range(NTAP):
            dy = t // K
            kp[dy:, t] = kb[:, t, :HSH - dy].transpose(1, 0, 2)

        tlb = np.zeros((NTAIL, 2 * CW + HSH), f16)
        xtv = tlb[:, 0:CW].reshape(NTAIL, C, W)
        ktv = tlb[:, CW:2 * CW].reshape(NTAIL, C, W)
        wtv = tlb[:, 2 * CW:]
        for j, (dy, p) in enumerate(COMBOS):
            for dx in range(K):
                xtv[j * K + dx] = blk[:, p + dy, dx:dx + W]
                ktv[j * K + dx] = kb[:, dy * K + dx, p]
                wtv[j * K + dx, p] = 1.0

        in_maps.append({"xe": xeb, "tl": tlb, "kp": kp})
    return in_maps


def kernel(x: np.ndarray, kernel: np.ndarray) -> np.ndarray:
    nc = get_nc()
    in_maps = _prep_shards(np.asarray(x), np.asarray(kernel))
    trace = bool(int(os.environ.get("KC_TRACE", "0")))
    res = run_bass_kernel_spmd(nc, in_maps, core_ids=list(range(NCORES)),
                               trace=trace)
    _NC_CACHE["last_results"] = res
    out = np.empty((N, C, H, W), np.float32)
    for core in range(NCORES):
        n, hb = divmod(core, 2)
        h0 = hb * HSH
        o = res.results[core]["out"]  # (128, C, W) fp16
        out[n, :, h0:h0 + HSH, :] = o.transpose(1, 0, 2).astype(np.float32)
    return out
